# revision 1
# baseline (speedup 1.0000x reference)
"""Local (banded) attention kernel for Trainium2, sharded over 8 NeuronCores.

Sharding: core c handles batch b=c//4 and heads 4*(c%4)..4*(c%4)+3.
Host pre-transposes x and weight slices; device does QKV projection,
banded attention (window 128 -> only tile-diagonal +/-1 blocks), and the
per-core slice of the output projection. Host sums the 4 partial outputs
per batch and adds the output bias.
"""

import ml_dtypes
import numpy as np

import concourse.bass as bass
import concourse.mybir as mybir
from concourse import bacc
from concourse.tile import TileContext
from concourse.bass_utils import run_bass_kernel_spmd
from concourse.masks import make_identity

B, N, E, H, DH, WIN = 2, 2048, 1024, 16, 64, 128
HPC = 4              # heads per core
SL = HPC * DH        # feature slice per core (256)
NT = N // 128        # 16 query/key tiles
F32 = mybir.dt.float32
F32R = mybir.dt.float32r
BF16 = mybir.dt.bfloat16
SCALE = 1.0 / 32.0   # 1/sqrt(E)
AUXW = 264           # aux rows: 0=bv, 1=ones, 2=zeros

_CACHED_NC = None


def _build_nc():
    nc = bacc.Bacc("TRN2", target_bir_lowering=False)

    xT_d = nc.dram_tensor("xT", [E, N], F32R, kind="ExternalInput")
    wqT_d = nc.dram_tensor("wqT", [E, SL], F32R, kind="ExternalInput")
    wkT_d = nc.dram_tensor("wkT", [E, SL], F32R, kind="ExternalInput")
    wvT_d = nc.dram_tensor("wvT", [E, SL], F32R, kind="ExternalInput")
    wpT_d = nc.dram_tensor("wpT", [SL, E], F32R, kind="ExternalInput")
    bq_d = nc.dram_tensor("bq", [SL], F32, kind="ExternalInput")
    bk_d = nc.dram_tensor("bk", [SL], F32, kind="ExternalInput")
    aux_d = nc.dram_tensor("aux", [3, AUXW], F32R, kind="ExternalInput")
    idr_d = nc.dram_tensor("idr", [128, 128], F32R, kind="ExternalInput")
    y_d = nc.dram_tensor("y", [N, E], F32, kind="ExternalOutput")

    KO = E // 128  # 8 contraction tiles

    with TileContext(nc) as tc:
        with (
            tc.tile_pool(name="const", bufs=1) as const,
            tc.tile_pool(name="persist", bufs=1) as persist,
            tc.tile_pool(name="io", bufs=3) as io,
            tc.tile_pool(name="small", bufs=6) as small,
            tc.tile_pool(name="strips", bufs=20) as strip_pool,
            tc.tile_pool(name="ps_mm", bufs=2, space="PSUM") as ps_mm,
            tc.tile_pool(name="ps_e", bufs=2, space="PSUM") as ps_e,
            tc.tile_pool(name="ps_u", bufs=2, space="PSUM") as ps_u,
            tc.tile_pool(name="ps_t", bufs=2, space="PSUM") as ps_t,
        ):
            # ---- small constants first (cheap DMAs) ----
            t_bv = const.tile([1, SL], F32R, name="t_bv")
            nc.sync.dma_start(t_bv[:], aux_d.ap()[0:1, :SL])
            t_ones = const.tile([1, AUXW], F32R, name="t_ones")
            nc.sync.dma_start(t_ones[:], aux_d.ap()[1:2, :])
            t_zero = const.tile([1, 128], F32R, name="t_zero")
            nc.sync.dma_start(t_zero[:], aux_d.ap()[2:3, :128])
            bv_row = t_bv[:]
            ones_row = t_ones[:, :128]
            zero_row = t_zero[:]
            rhs260 = t_ones[:, :HPC * (DH + 1)]
            bq_col = const.tile([128, 2], F32)
            nc.sync.dma_start(bq_col[:], bq_d.ap().rearrange("(g p) -> p g", p=128))
            bk_col = const.tile([128, 2], F32)
            nc.sync.dma_start(bk_col[:], bk_d.ap().rearrange("(g p) -> p g", p=128))

            ident = const.tile([128, 128], BF16)
            identr = const.tile([128, 128], F32R)
            nc.sync.dma_start(identr[:], idr_d.ap())
            make_identity(nc, ident[:])
            # 3-block band mask [U | ones | L] for the strip of a key tile
            mask3 = const.tile([128, 384], BF16)
            nc.gpsimd.memset(mask3[:], 1.0)
            nc.gpsimd.affine_select(
                out=mask3[:, 0:128], in_=mask3[:, 0:128],
                compare_op=mybir.AluOpType.is_ge, fill=0.0, base=0,
                pattern=[[1, 128]], channel_multiplier=-1)  # keep c >= p
            nc.gpsimd.affine_select(
                out=mask3[:, 256:384], in_=mask3[:, 256:384],
                compare_op=mybir.AluOpType.is_ge, fill=0.0, base=0,
                pattern=[[-1, 128]], channel_multiplier=1)  # keep c <= p

            # ---- weights before x so compute can start early ----
            wq_sb = persist.tile([128, KO, SL], F32R)
            nc.sync.dma_start(wq_sb[:], wqT_d.ap().rearrange("(ko p) m -> p ko m", p=128))
            wk_sb = persist.tile([128, KO, SL], F32R)
            wv_sb = persist.tile([128, KO, SL], F32R)
            wp_sb = persist.tile([128, 2, E], F32R)

            xT_sb = persist.tile([128, KO, N], F32R)
            xT_ap = xT_d.ap().rearrange("(ko p) n -> p ko n", p=128)
            for c8 in range(8):
                s = slice(c8 * (N // 8), (c8 + 1) * (N // 8))
                nc.sync.dma_start(xT_sb[:, :, s], xT_ap[:, :, s])
                if c8 == 1:
                    nc.sync.dma_start(
                        wk_sb[:], wkT_d.ap().rearrange("(ko p) m -> p ko m", p=128))
                    nc.sync.dma_start(
                        wv_sb[:], wvT_d.ap().rearrange("(ko p) m -> p ko m", p=128))
                if c8 == 3:
                    nc.sync.dma_start(
                        wp_sb[:], wpT_d.ap().rearrange("(g p) f -> p g f", p=128))
            NCH = 4
            CW = N // NCH  # 512

            # ---- projection outputs ----
            qT = [persist.tile([128, N], F32R, name=f"qT{g}", tag=f"qT{g}")
                  for g in range(2)]
            kT = [persist.tile([128, N], F32R, name=f"kT{g}", tag=f"kT{g}")
                  for g in range(2)]
            vaug = persist.tile([128, NT, HPC, DH + 1], BF16)
            nc.gpsimd.memset(vaug[:, :, :, DH], 1.0)
            attT = [persist.tile([128, N], F32R, name=f"attT{g}", tag=f"attT{g}")
                    for g in range(2)]

            # ---- phase 2: QKV per x-chunk (called interleaved below) ----
            def emit_qkv_chunk(ch):
                cs = slice(ch * CW, (ch + 1) * CW)
                for w_sb, out_t, b_col in ((wq_sb, qT, bq_col), (wk_sb, kT, bk_col)):
                    for g in range(2):
                        ps = ps_mm.tile([128, 512], F32, tag="mm", name="ps_qk")
                        for kt in range(KO):
                            nc.tensor.matmul(
                                ps[:],
                                lhsT=w_sb[:, kt, g * 128:(g + 1) * 128],
                                rhs=xT_sb[:, kt, cs],
                                start=(kt == 0), stop=(kt == KO - 1))
                        nc.scalar.activation(
                            out_t[g][:, cs], ps[:],
                            mybir.ActivationFunctionType.Identity,
                            bias=b_col[:, g:g + 1])
                for nt in range(ch * NCH, (ch + 1) * NCH):
                    ps = ps_mm.tile([128, 512], F32, tag="mm", name="ps_v")
                    psv = ps[:, :SL]
                    rs = slice(nt * 128, (nt + 1) * 128)
                    for kt in range(KO):
                        nc.tensor.matmul(
                            psv, lhsT=xT_sb[:, kt, rs], rhs=wv_sb[:, kt, :],
                            start=(kt == 0), stop=False)
                    nc.tensor.matmul(
                        psv, lhsT=ones_row, rhs=bv_row,
                        start=False, stop=True)
                    nc.vector.tensor_copy(
                        vaug[:, nt, :, :DH],
                        psv.rearrange("p (h d) -> p h d", d=DH))

            # ---- phase 3+4: banded attention, fused projection + store ----
            # interleaved with QKV chunks: strip kj needs q cols up to
            # 128*(kj+2) <= 512*(ch+1)  =>  kj <= 4*ch + 2
            strips = {}

            def emit_strip(h, kj):
                g, po = h // 2, (h % 2) * 64
                qh = qT[g][po:po + 64, :]
                kh = kT[g][po:po + 64, :]
                lo, hi = max(0, kj - 1), min(NT - 1, kj + 1)
                w = (hi - lo + 1) * 128
                moff = 0 if lo == kj - 1 else 128
                pe = ps_e.tile([128, 384], F32, tag="pe", name="pe")
                nc.tensor.matmul(
                    pe[:, :w],
                    lhsT=kh[:, kj * 128:(kj + 1) * 128],
                    rhs=qh[:, lo * 128:(hi + 1) * 128],
                    start=True, stop=True)
                st = strip_pool.tile([128, 384], BF16, tag="strip", name="st")
                nc.scalar.activation(
                    st[:, :w], pe[:, :w],
                    mybir.ActivationFunctionType.Exp, scale=SCALE)
                nc.vector.tensor_mul(
                    st[:, :w], st[:, :w], mask3[:, moff:moff + w])
                strips[(h, kj)] = (st, lo)

            def process_tile(t):
                ts_ = slice(t * 128, (t + 1) * 128)
                ks = [k for k in (t - 1, t, t + 1) if 0 <= k < NT]
                pu = ps_u.tile([128, HPC, DH + 1], F32, tag="pu", name="pu")
                # zero-fill the whole bank so the 12 AV matmuls accumulate
                # order-independently (has_written set everywhere once)
                nc.tensor.matmul(
                    pu[:], lhsT=zero_row, rhs=rhs260,
                    start=True, stop=False, skip_group_check=True)
                for h in range(HPC):
                    for i, k2 in enumerate(ks):
                        st, lo2 = strips[(h, k2)]
                        col = (t - lo2) * 128
                        nc.tensor.matmul(
                            pu[:, h, :], lhsT=st[:, col:col + 128],
                            rhs=vaug[:, k2, h, :],
                            start=False,
                            stop=(h == HPC - 1 and i == len(ks) - 1),
                            skip_group_check=True)
                rec = small.tile([128, HPC], F32, tag="rec", name="rec")
                nc.vector.reciprocal(rec[:], pu[:, :, DH])
                ao = small.tile([128, HPC, DH], F32R, tag="ao", name="ao")
                for h in range(HPC):
                    nc.vector.tensor_scalar_mul(
                        ao[:, h, :], pu[:, h, :DH], rec[:, h:h + 1])
                for g in range(2):
                    pt = ps_t.tile([128, 128], F32R, tag="pt", name="pt")
                    nc.tensor.transpose(
                        pt[:], ao[:, 2 * g:2 * g + 2, :], identr[:])
                    if g == 0:
                        nc.scalar.activation(
                            attT[g][:, ts_], pt[:],
                            mybir.ActivationFunctionType.Copy)
                    else:
                        nc.vector.tensor_copy(attT[g][:, ts_], pt[:])
                # fused output projection for this token tile
                y_sb = io.tile([128, E], F32, tag="y", name="y_sb")
                for fc in range(2):
                    ps = ps_mm.tile([128, 512], F32, tag="mm", name="ps_y")
                    fs = slice(fc * 512, (fc + 1) * 512)
                    for g in range(2):
                        nc.tensor.matmul(
                            ps[:],
                            lhsT=attT[g][:, ts_],
                            rhs=wp_sb[:, g, fs],
                            start=(g == 0), stop=(g == 1))
                    if fc == 0:
                        nc.scalar.activation(
                            y_sb[:, fs], ps[:],
                            mybir.ActivationFunctionType.Copy)
                    else:
                        nc.vector.tensor_copy(y_sb[:, fs], ps[:])
                nc.sync.dma_start(y_d[ts_, :], y_sb[:])

            for ch in range(NCH):
                emit_qkv_chunk(ch)
            LEAD = 2
            for kj in range(NT):
                for h in range(HPC):
                    emit_strip(h, kj)
                if kj >= LEAD:
                    process_tile(kj - LEAD)
            for t in range(NT - LEAD, NT):
                process_tile(t)

    nc.compile()
    return nc


def _get_nc():
    global _CACHED_NC
    if _CACHED_NC is None:
        _CACHED_NC = _build_nc()
    return _CACHED_NC


def kernel(x, Wq, bq, Wk, bk, Wv, bv, Wp, bp):
    nc = _get_nc()
    x = np.asarray(x, np.float32)
    xTs = [np.ascontiguousarray(x[b].T) for b in range(B)]
    in_maps = []
    for c in range(8):
        b, gq = c // 4, c % 4
        sl = slice(SL * gq, SL * (gq + 1))
        aux = np.zeros((3, AUXW), np.float32)
        aux[0, :SL] = np.asarray(bv, np.float32)[sl]
        aux[1, :] = 1.0
        in_maps.append({
            "xT": xTs[b],
            "wqT": np.ascontiguousarray(np.asarray(Wq, np.float32)[sl].T),
            "wkT": np.ascontiguousarray(np.asarray(Wk, np.float32)[sl].T),
            "wvT": np.ascontiguousarray(np.asarray(Wv, np.float32)[sl].T),
            "wpT": np.ascontiguousarray(np.asarray(Wp, np.float32)[:, sl].T),
            "bq": np.ascontiguousarray(np.asarray(bq, np.float32)[sl]),
            "bk": np.ascontiguousarray(np.asarray(bk, np.float32)[sl]),
            "aux": aux,
            "idr": np.eye(128, dtype=np.float32),
        })
    res = run_bass_kernel_spmd(nc, in_maps, core_ids=list(range(8)))
    ys = [res.results[c]["y"] for c in range(8)]
    bp = np.asarray(bp, np.float32)
    y = np.stack([
        ys[0] + ys[1] + ys[2] + ys[3],
        ys[4] + ys[5] + ys[6] + ys[7],
    ]).astype(np.float32) + bp[None, None, :]
    return y.astype(np.float32)



# revision 9
# speedup vs baseline: 1.1855x; 1.1855x over previous
"""Local (banded) attention kernel for Trainium2, sharded over 8 NeuronCores.

Sharding: core c handles batch b=c//4 and heads 4*(c%4)..4*(c%4)+3.
Q/K projections and QK^T run as fp8 DoubleRow matmuls (host pre-quantizes
x and the scaled Q/K weight slices, permuted so each head's 64-dim split
lands as [32 partitions x 2 DR slots]).  The band mask is accumulated into
the energy PSUM via tiny fp8e5 DoubleRow matmuls (identity stationary x
precomputed -57344 panels), so exp() needs no separate mask pass.  V and
output projections run in f16; y partials stream out in f16 and the host
sums the 4 partials per batch in f32 and adds the output bias.
"""

import ml_dtypes
import numpy as np

import concourse.bass as bass
import concourse.mybir as mybir
from concourse import bacc
from concourse.tile import TileContext
from concourse.bass_utils import run_bass_kernel_spmd
from concourse.masks import make_identity

B, N, E, H, DH, WIN = 2, 2048, 1024, 16, 64, 128
HPC = 4              # heads per core
SL = HPC * DH        # feature slice per core (256)
NT = N // 128        # 16 query/key tiles
F32 = mybir.dt.float32
F16 = mybir.dt.float16
BF16 = mybir.dt.bfloat16
F8 = mybir.dt.float8e4
F8E5 = mybir.dt.float8e5
SCALE = 1.0 / 32.0   # 1/sqrt(E)
WSCALE = 32.0        # Q/K weights are shipped as fp8(32*W); undone in copies
MASKVAL = -57344.0   # exactly representable in e5m2; /32 => -1792 pre-exp
AUXW = 264           # aux rows: 0=bv, 1=ones, 2=zeros
KO = E // 128        # 8 contraction tiles
KP = KO // 2         # 4 DoubleRow contraction-pair tiles
DR = mybir.MatmulPerfMode.DoubleRow

_CACHED_NC = None


def _build_nc():
    nc = bacc.Bacc("TRN2", target_bir_lowering=False)

    x8_d = nc.dram_tensor("x8", [E, N], F8, kind="ExternalInput")
    xT_d = nc.dram_tensor("xT", [E, N], F16, kind="ExternalInput")
    wq8_d = nc.dram_tensor("wq8", [E, 2, 128], F8, kind="ExternalInput")
    wk8_d = nc.dram_tensor("wk8", [E, 2, 128], F8, kind="ExternalInput")
    wv_d = nc.dram_tensor("wv", [E, SL], F16, kind="ExternalInput")
    wp_d = nc.dram_tensor("wp", [SL, E], F16, kind="ExternalInput")
    bqk_d = nc.dram_tensor("bqk", [128, 4], F32, kind="ExternalInput")
    aux_d = nc.dram_tensor("aux", [3, AUXW], BF16, kind="ExternalInput")
    msk_d = nc.dram_tensor("msk", [128, 2, 256], F8E5, kind="ExternalInput")
    idz_d = nc.dram_tensor("idz", [128, 2, 128], F8E5, kind="ExternalInput")
    y_d = nc.dram_tensor("y", [N, E], F16, kind="ExternalOutput")

    with TileContext(nc) as tc:
        with (
            tc.tile_pool(name="const", bufs=1) as const,
            tc.tile_pool(name="persist", bufs=1) as persist,
            tc.tile_pool(name="io", bufs=3) as io,
            tc.tile_pool(name="small", bufs=6) as small,
            tc.tile_pool(name="att2p", bufs=3) as att2p,
            tc.tile_pool(name="strips", bufs=20) as strip_pool,
            tc.tile_pool(name="ps_mm", bufs=2, space="PSUM") as ps_mm,
            tc.tile_pool(name="ps_e", bufs=2, space="PSUM") as ps_e,
            tc.tile_pool(name="ps_u", bufs=2, space="PSUM") as ps_u,
            tc.tile_pool(name="ps_t", bufs=2, space="PSUM") as ps_t,
        ):
            # ---- small constants first (cheap DMAs) ----
            t_bv = const.tile([1, SL], BF16, name="t_bv")
            nc.sync.dma_start(t_bv[:], aux_d.ap()[0:1, :SL])
            t_ones = const.tile([1, AUXW], BF16, name="t_ones")
            nc.sync.dma_start(t_ones[:], aux_d.ap()[1:2, :])
            t_zero = const.tile([1, 128], BF16, name="t_zero")
            nc.sync.dma_start(t_zero[:], aux_d.ap()[2:3, :128])
            bv_row = t_bv[:]
            ones_row = t_ones[:, :128]
            zero_row = t_zero[:]
            rhs260 = t_ones[:, :HPC * (DH + 1)]
            bqk = const.tile([128, 4], F32)
            nc.sync.dma_start(bqk[:], bqk_d.ap())
            msk = const.tile([128, 2, 256], F8E5)
            nc.sync.dma_start(msk[:], msk_d.ap())
            idz = const.tile([128, 2, 128], F8E5)
            nc.sync.dma_start(idz[:], idz_d.ap())
            ident = const.tile([128, 128], BF16)
            make_identity(nc, ident[:])

            # ---- weights before x so compute can start early ----
            wq_sb = persist.tile([128, KP, 2, 2, 128], F8)
            nc.sync.dma_start(
                wq_sb[:], wq8_d.ap().rearrange(
                    "(kp dr p) i m -> p kp dr i m", p=128, dr=2))
            wk_sb = persist.tile([128, KP, 2, 2, 128], F8)
            nc.sync.dma_start(
                wk_sb[:], wk8_d.ap().rearrange(
                    "(kp dr p) i m -> p kp dr i m", p=128, dr=2))
            wv_sb = persist.tile([128, KO, SL], F16)
            nc.sync.dma_start(
                wv_sb[:], wv_d.ap().rearrange("(ko p) m -> p ko m", p=128))
            wp_sb = persist.tile([128, 2, E], F16)
            nc.sync.dma_start(
                wp_sb[:], wp_d.ap().rearrange("(g p) f -> p g f", p=128))

            x8_sb = persist.tile([128, KO, N], F8)
            xT_sb = persist.tile([128, KO, N], F16)
            x8_ap = x8_d.ap().rearrange("(ko p) n -> p ko n", p=128)
            xT_ap = xT_d.ap().rearrange("(ko p) n -> p ko n", p=128)
            NCH = 4
            CW = N // NCH  # 512
            for c4 in range(NCH):
                s = slice(c4 * CW, (c4 + 1) * CW)
                nc.sync.dma_start(x8_sb[:, :, s], x8_ap[:, :, s])
            for c4 in range(NCH):
                s = slice(c4 * CW, (c4 + 1) * CW)
                nc.sync.dma_start(xT_sb[:, :, s], xT_ap[:, :, s])

            # ---- projection outputs ----
            # q8/k8 layout (hw only allows AP base partitions 0/32/64):
            #   head 0: partitions  0-31, slots 0/1 = dh halves  (DoubleRow)
            #   head 1: partitions 32-63, slots 0/1 = dh halves  (DoubleRow)
            #   head 2: partitions 64-127, slot 0 = full dh      (plain fp8)
            #   head 3: partitions 64-127, slot 1 = full dh      (plain fp8)
            q8 = persist.tile([128, 2, N], F8, name="q8", tag="q8")
            k8 = persist.tile([128, 2, N], F8, name="k8", tag="k8")
            vaug = persist.tile([128, NT, HPC, DH + 1], BF16)
            nc.gpsimd.memset(vaug[:, :, :, DH], 1.0)

            # ---- phase 2: QKV per x-chunk ----
            def emit_qkv_chunk(ch):
                for w_sb, out_t, bc in ((wq_sb, q8, 0), (wk_sb, k8, 2)):
                    for c2 in range(2):
                        cs = slice(ch * CW + c2 * 256, ch * CW + c2 * 256 + 256)
                        ps = ps_mm.tile([128, 2, 256], F32, tag="mm", name="ps_qk")
                        for i in range(2):
                            for kp in range(KP):
                                nc.tensor.matmul(
                                    ps[:, i, :],
                                    lhsT=w_sb[:, kp, :, i, :],
                                    rhs=x8_sb[:, 2 * kp:2 * kp + 2, cs],
                                    start=(kp == 0), stop=(kp == KP - 1),
                                    perf_mode=DR)
                        nc.scalar.activation(
                            out_t[:, 0, cs], ps[:, 0, :],
                            mybir.ActivationFunctionType.Identity,
                            scale=1.0 / WSCALE, bias=bqk[:, bc:bc + 1])
                        nc.vector.tensor_scalar(
                            out_t[:, 1, cs], ps[:, 1, :],
                            1.0 / WSCALE, bqk[:, bc + 1:bc + 2],
                            mybir.AluOpType.mult, mybir.AluOpType.add)
                for nt in range(ch * NCH, (ch + 1) * NCH):
                    ps = ps_mm.tile([128, 2, 256], F32, tag="mm", name="ps_v")
                    psv = ps[:, 0, :]
                    rs = slice(nt * 128, (nt + 1) * 128)
                    for kt in range(KO):
                        nc.tensor.matmul(
                            psv, lhsT=xT_sb[:, kt, rs], rhs=wv_sb[:, kt, :],
                            start=(kt == 0), stop=False)
                    nc.tensor.matmul(
                        psv, lhsT=ones_row, rhs=bv_row,
                        start=False, stop=True)
                    nc.vector.tensor_copy(
                        vaug[:, nt, :, :DH],
                        psv.rearrange("p (h d) -> p h d", d=DH))

            # ---- phase 3+4: banded attention, fused projection + store ----
            strips = {}

            def emit_strip(h, kj):
                lo, hi = max(0, kj - 1), min(NT - 1, kj + 1)
                w = (hi - lo + 1) * 128
                kjs = slice(kj * 128, (kj + 1) * 128)
                sps = slice(lo * 128, (hi + 1) * 128)
                pe = ps_e.tile([128, 384], F32, tag="pe", name="pe")
                if h < 2:
                    hs = slice(32 * h, 32 * h + 32)
                    nc.tensor.matmul(
                        pe[:, :w], lhsT=k8[hs, :, kjs], rhs=q8[hs, :, sps],
                        start=True, stop=False, perf_mode=DR,
                        skip_group_check=True)
                else:
                    i = h - 2
                    nc.tensor.matmul(
                        pe[:, :w], lhsT=k8[64:128, i, kjs],
                        rhs=q8[64:128, i, sps],
                        start=True, stop=False, skip_group_check=True)
                # band-mask accumulation: identity (e5m2) x mask panels
                panels = []
                if kj > 0:   # U panel sits where queries tile kj-1 lives
                    panels.append(((kj - 1 - lo) * 128, 0))
                if kj < NT - 1:  # L panel at queries tile kj+1
                    panels.append(((kj + 1 - lo) * 128, 128))
                for n_, (po, mo) in enumerate(panels):
                    nc.tensor.matmul(
                        pe[:, po:po + 128],
                        lhsT=idz[:],
                        rhs=msk[:, :, mo:mo + 128],
                        start=False, stop=(n_ == len(panels) - 1),
                        perf_mode=DR, skip_group_check=True)
                st = strip_pool.tile([128, 384], BF16, tag="strip", name="st")
                nc.scalar.activation(
                    st[:, :w], pe[:, :w],
                    mybir.ActivationFunctionType.Exp, scale=SCALE)
                strips[(h, kj)] = (st, lo)

            def process_tile(t):
                ts_ = slice(t * 128, (t + 1) * 128)
                ks = [k for k in (t - 1, t, t + 1) if 0 <= k < NT]
                pu = ps_u.tile([128, HPC, DH + 1], F32, tag="pu", name="pu")
                # zero-fill the whole bank so the 12 AV matmuls accumulate
                # order-independently (has_written set everywhere once)
                nc.tensor.matmul(
                    pu[:], lhsT=zero_row, rhs=rhs260,
                    start=True, stop=False, skip_group_check=True)
                for h in range(HPC):
                    for i, k2 in enumerate(ks):
                        st, lo2 = strips[(h, k2)]
                        col = (t - lo2) * 128
                        nc.tensor.matmul(
                            pu[:, h, :], lhsT=st[:, col:col + 128],
                            rhs=vaug[:, k2, h, :],
                            start=False,
                            stop=(h == HPC - 1 and i == len(ks) - 1),
                            skip_group_check=True)
                rec = small.tile([128, HPC], F32, tag="rec", name="rec")
                nc.vector.reciprocal(rec[:], pu[:, :, DH])
                ao = small.tile([128, HPC, DH], BF16, tag="ao", name="ao")
                for h in range(2):
                    nc.vector.tensor_scalar_mul(
                        ao[:, h, :], pu[:, h, :DH], rec[:, h:h + 1])
                for h in range(2, HPC):
                    nc.scalar.activation(
                        ao[:, h, :], pu[:, h, :DH],
                        mybir.ActivationFunctionType.Identity,
                        scale=rec[:, h:h + 1])
                att2 = att2p.tile([128, 2, 128], BF16, tag="att2", name="att2")
                for g in range(2):
                    pt = ps_t.tile([128, 128], BF16, tag="pt", name="pt")
                    nc.tensor.transpose(
                        pt[:], ao[:, 2 * g:2 * g + 2, :], ident[:])
                    if g == 0:
                        nc.scalar.activation(
                            att2[:, g, :], pt[:],
                            mybir.ActivationFunctionType.Copy)
                    else:
                        nc.vector.tensor_copy(att2[:, g, :], pt[:])
                # fused output projection for this token tile
                y_sb = io.tile([128, E], F16, tag="y", name="y_sb")
                for fc in range(2):
                    ps = ps_mm.tile([128, 2, 256], F32, tag="mm", name="ps_y")
                    psy = ps.rearrange("p a b -> p (a b)")
                    fs = slice(fc * 512, (fc + 1) * 512)
                    for g in range(2):
                        nc.tensor.matmul(
                            psy,
                            lhsT=att2[:, g, :],
                            rhs=wp_sb[:, g, fs],
                            start=(g == 0), stop=(g == 1))
                    if fc == 0:
                        nc.scalar.activation(
                            y_sb[:, fs], psy,
                            mybir.ActivationFunctionType.Copy)
                    else:
                        nc.vector.tensor_copy(y_sb[:, fs], psy)
                nc.sync.dma_start(y_d[ts_, :], y_sb[:])

            for ch in range(NCH):
                emit_qkv_chunk(ch)
            LEAD = 2
            for kj in range(NT):
                for h in range(HPC):
                    emit_strip(h, kj)
                if kj >= LEAD:
                    process_tile(kj - LEAD)
            for t in range(NT - LEAD, NT):
                process_tile(t)

    nc.compile()
    return nc


def _get_nc():
    global _CACHED_NC
    if _CACHED_NC is None:
        _CACHED_NC = _build_nc()
    return _CACHED_NC


def _prep_core(x_b, Wq, bq, Wk, bk, Wv, Wp, gq):
    f8 = ml_dtypes.float8_e4m3
    f8e5 = ml_dtypes.float8_e5m2
    sl = slice(SL * gq, SL * (gq + 1))
    xT = np.ascontiguousarray(x_b.T).astype(np.float32)

    # feature index (within this core's 256-slice) at (slot i, partition m):
    #   m<32: head0 dh=32i+m; 32<=m<64: head1 dh=32i+(m-32);
    #   m>=64: head (2+i), dh=m-64
    fidx = np.zeros((2, 128), np.int64)
    m = np.arange(128)
    for i in range(2):
        fidx[i, :32] = 32 * i + m[:32]
        fidx[i, 32:64] = 64 + 32 * i + (m[32:64] - 32)
        fidx[i, 64:] = 64 * (2 + i) + (m[64:] - 64)

    def qk_weight(W):
        w = np.ascontiguousarray(W[sl].T).astype(np.float32) * WSCALE
        return w[:, fidx].astype(f8)  # [E, 2, 128]

    def qk_bias(b):
        return np.asarray(b, np.float32)[sl][fidx]  # [2, 128]

    bq2, bk2 = qk_bias(bq), qk_bias(bk)
    bqk = np.stack([bq2[0], bq2[1], bk2[0], bk2[1]], axis=1)  # [128, 4]

    # band-mask panels: U keeps qcol >= p, L keeps qcol <= p
    pi = np.arange(128)
    msk = np.zeros((128, 2, 256), np.float32)
    msk[:, 0, :128] = np.where(pi[None, :] >= pi[:, None], 0.0, MASKVAL)
    msk[:, 0, 128:] = np.where(pi[None, :] <= pi[:, None], 0.0, MASKVAL)
    idz = np.zeros((128, 2, 128), np.float32)
    idz[:, 0, :] = np.eye(128, dtype=np.float32)

    return {
        "x8": xT.astype(f8),
        "xT": xT.astype(np.float16),
        "wq8": qk_weight(Wq),
        "wk8": qk_weight(Wk),
        "wv": np.ascontiguousarray(np.asarray(Wv, np.float32)[sl].T).astype(
            np.float16),
        "wp": np.ascontiguousarray(np.asarray(Wp, np.float32)[:, sl].T).astype(
            np.float16),
        "bqk": np.ascontiguousarray(bqk),
        "msk": msk.astype(f8e5),
        "idz": idz.astype(f8e5),
    }


def kernel(x, Wq, bq, Wk, bk, Wv, bv, Wp, bp):
    nc = _get_nc()
    x = np.asarray(x, np.float32)
    in_maps = []
    for c in range(8):
        b, gq = c // 4, c % 4
        m = _prep_core(x[b], np.asarray(Wq, np.float32), bq,
                       np.asarray(Wk, np.float32), bk,
                       np.asarray(Wv, np.float32),
                       np.asarray(Wp, np.float32), gq)
        sl = slice(SL * gq, SL * (gq + 1))
        aux = np.zeros((3, AUXW), np.float32)
        aux[0, :SL] = np.asarray(bv, np.float32)[sl]
        aux[1, :] = 1.0
        m["aux"] = aux.astype(ml_dtypes.bfloat16)
        in_maps.append(m)
    res = run_bass_kernel_spmd(nc, in_maps, core_ids=list(range(8)))
    ys = [res.results[c]["y"].astype(np.float32) for c in range(8)]
    bp = np.asarray(bp, np.float32)
    y = np.stack([
        ys[0] + ys[1] + ys[2] + ys[3],
        ys[4] + ys[5] + ys[6] + ys[7],
    ]) + bp[None, None, :]
    return y.astype(np.float32)


# revision 14
# speedup vs baseline: 1.2595x; 1.0624x over previous
"""Local (banded) attention kernel for Trainium2, sharded over 8 NeuronCores.

Sharding: core c handles batch b=c//4 and heads 4*(c%4)..4*(c%4)+3.
Q/K projections and QK^T run as fp8 DoubleRow matmuls (host pre-quantizes
x and the scaled Q/K weight slices, permuted so each head's 64-dim split
lands as [32 partitions x 2 DR slots]).  The band mask is accumulated into
the energy PSUM via tiny fp8e5 DoubleRow matmuls (identity stationary x
precomputed -57344 panels), so exp() needs no separate mask pass.  V and
output projections run in f16; y partials stream out in f16 and the host
sums the 4 partials per batch in f32 and adds the output bias.
"""

import ml_dtypes
import numpy as np

import concourse.bass as bass
import concourse.mybir as mybir
from concourse import bacc
from concourse.tile import TileContext
from concourse.bass_utils import run_bass_kernel_spmd
from concourse.masks import make_identity

B, N, E, H, DH, WIN = 2, 2048, 1024, 16, 64, 128
HPC = 4              # heads per core
SL = HPC * DH        # feature slice per core (256)
NT = N // 128        # 16 query/key tiles
F32 = mybir.dt.float32
F16 = mybir.dt.float16
BF16 = mybir.dt.bfloat16
F8 = mybir.dt.float8e4
F8E5 = mybir.dt.float8e5
SCALE = 1.0 / 32.0   # 1/sqrt(E)
WSCALE = 32.0        # Q/K weights are shipped as fp8(32*W); undone in copies
MASKVAL = -57344.0   # exactly representable in e5m2; /32 => -1792 pre-exp
AUXW = 264           # aux rows: 0=bv, 1=ones, 2=zeros
KO = E // 128        # 8 contraction tiles
KP = KO // 2         # 4 DoubleRow contraction-pair tiles
DR = mybir.MatmulPerfMode.DoubleRow

_CACHED_NC = None


def _build_nc():
    nc = bacc.Bacc("TRN2", target_bir_lowering=False)

    x8_d = nc.dram_tensor("x8", [E, N], F8, kind="ExternalInput")
    xT_d = nc.dram_tensor("xT", [E, N], F16, kind="ExternalInput")
    wq8_d = nc.dram_tensor("wq8", [E, 2, 128], F8, kind="ExternalInput")
    wk8_d = nc.dram_tensor("wk8", [E, 2, 128], F8, kind="ExternalInput")
    wv_d = nc.dram_tensor("wv", [E, SL], F16, kind="ExternalInput")
    wp_d = nc.dram_tensor("wp", [SL, E], F16, kind="ExternalInput")
    bqk_d = nc.dram_tensor("bqk", [128, 4], F32, kind="ExternalInput")
    aux_d = nc.dram_tensor("aux", [3, AUXW], BF16, kind="ExternalInput")
    msk_d = nc.dram_tensor("msk", [128, 2, 256], F8E5, kind="ExternalInput")
    idz_d = nc.dram_tensor("idz", [128, 2, 128], F8E5, kind="ExternalInput")
    y_d = nc.dram_tensor("y", [N, E], F16, kind="ExternalOutput")

    with TileContext(nc) as tc:
        with (
            tc.tile_pool(name="const", bufs=1) as const,
            tc.tile_pool(name="persist", bufs=1) as persist,
            tc.tile_pool(name="io", bufs=3) as io,
            tc.tile_pool(name="small", bufs=6) as small,
            tc.tile_pool(name="att2p", bufs=3) as att2p,
            tc.tile_pool(name="strips", bufs=20) as strip_pool,
            tc.tile_pool(name="ps_mm", bufs=2, space="PSUM") as ps_mm,
            tc.tile_pool(name="ps_e", bufs=2, space="PSUM") as ps_e,
            tc.tile_pool(name="ps_u", bufs=2, space="PSUM") as ps_u,
            tc.tile_pool(name="ps_t", bufs=2, space="PSUM") as ps_t,
        ):
            # ---- small constants first (cheap DMAs) ----
            t_bv = const.tile([1, SL], BF16, name="t_bv")
            nc.sync.dma_start(t_bv[:], aux_d.ap()[0:1, :SL])
            t_ones = const.tile([1, AUXW], BF16, name="t_ones")
            nc.sync.dma_start(t_ones[:], aux_d.ap()[1:2, :])
            t_zero = const.tile([1, 128], BF16, name="t_zero")
            nc.sync.dma_start(t_zero[:], aux_d.ap()[2:3, :128])
            bv_row = t_bv[:]
            ones_row = t_ones[:, :128]
            zero_row = t_zero[:]
            rhs260 = t_ones[:, :HPC * (DH + 1)]
            bqk = const.tile([128, 4], F32)
            nc.sync.dma_start(bqk[:], bqk_d.ap())
            msk = const.tile([128, 2, 256], F8E5)
            nc.sync.dma_start(msk[:], msk_d.ap())
            idz = const.tile([128, 2, 128], F8E5)
            nc.sync.dma_start(idz[:], idz_d.ap())
            ident = const.tile([128, 128], BF16)
            make_identity(nc, ident[:])

            # ---- weights before x so compute can start early ----
            wq_sb = persist.tile([128, KP, 2, 2, 128], F8)
            nc.sync.dma_start(
                wq_sb[:], wq8_d.ap().rearrange(
                    "(kp dr p) i m -> p kp dr i m", p=128, dr=2))
            wk_sb = persist.tile([128, KP, 2, 2, 128], F8)
            nc.sync.dma_start(
                wk_sb[:], wk8_d.ap().rearrange(
                    "(kp dr p) i m -> p kp dr i m", p=128, dr=2))
            x8_sb = persist.tile([128, KO, N], F8)
            xT_sb = persist.tile([128, KO, N], F16)
            wv_sb = persist.tile([128, KO, SL], F16)
            wp_sb = persist.tile([128, 2, E], F16)
            x8_ap = x8_d.ap().rearrange("(ko p) n -> p ko n", p=128)
            xT_ap = xT_d.ap().rearrange("(ko p) n -> p ko n", p=128)
            NCH = 4
            CW = N // NCH  # 512
            # DMA order: x8 chunk 0 first so Q/K proj starts early, then wv
            # + xT chunk 0 for the V projection, wp, then remaining chunks.
            s0 = slice(0, CW)
            nc.sync.dma_start(x8_sb[:, :, s0], x8_ap[:, :, s0])
            nc.sync.dma_start(
                wv_sb[:], wv_d.ap().rearrange("(ko p) m -> p ko m", p=128))
            nc.sync.dma_start(xT_sb[:, :, s0], xT_ap[:, :, s0])
            nc.sync.dma_start(
                wp_sb[:], wp_d.ap().rearrange("(g p) f -> p g f", p=128))
            for c4 in range(1, NCH):
                s = slice(c4 * CW, (c4 + 1) * CW)
                nc.sync.dma_start(x8_sb[:, :, s], x8_ap[:, :, s])
                nc.sync.dma_start(xT_sb[:, :, s], xT_ap[:, :, s])

            # ---- projection outputs ----
            # q8/k8 layout (hw only allows AP base partitions 0/32/64):
            #   head 0: partitions  0-31, slots 0/1 = dh halves  (DoubleRow)
            #   head 1: partitions 32-63, slots 0/1 = dh halves  (DoubleRow)
            #   head 2: partitions 64-127, slot 0 = full dh      (plain fp8)
            #   head 3: partitions 64-127, slot 1 = full dh      (plain fp8)
            q8 = persist.tile([128, 2, N], F8, name="q8", tag="q8")
            k8 = persist.tile([128, 2, N], F8, name="k8", tag="k8")
            vaug = persist.tile([128, NT, HPC, DH + 1], BF16)
            nc.gpsimd.memset(vaug[:, :, :, DH], 1.0)

            # ---- phase 2: QKV per x-chunk ----
            def emit_qkv_chunk(ch):
                for w_sb, out_t, bc in ((wq_sb, q8, 0), (wk_sb, k8, 2)):
                    for c2 in range(2):
                        cs = slice(ch * CW + c2 * 256, ch * CW + c2 * 256 + 256)
                        ps = ps_mm.tile([128, 2, 256], F32, tag="mm", name="ps_qk")
                        for i in range(2):
                            for kp in range(KP):
                                nc.tensor.matmul(
                                    ps[:, i, :],
                                    lhsT=w_sb[:, kp, :, i, :],
                                    rhs=x8_sb[:, 2 * kp:2 * kp + 2, cs],
                                    start=(kp == 0), stop=(kp == KP - 1),
                                    perf_mode=DR)
                        nc.scalar.activation(
                            out_t[:, 0, cs], ps[:, 0, :],
                            mybir.ActivationFunctionType.Identity,
                            scale=1.0 / WSCALE, bias=bqk[:, bc:bc + 1])
                        nc.vector.tensor_scalar(
                            out_t[:, 1, cs], ps[:, 1, :],
                            1.0 / WSCALE, bqk[:, bc + 1:bc + 2],
                            mybir.AluOpType.mult, mybir.AluOpType.add)
                for nt in range(ch * NCH, (ch + 1) * NCH):
                    ps = ps_mm.tile([128, 2, 256], F32, tag="mm", name="ps_v")
                    psv = ps[:, 0, :]
                    rs = slice(nt * 128, (nt + 1) * 128)
                    for kt in range(KO):
                        nc.tensor.matmul(
                            psv, lhsT=xT_sb[:, kt, rs], rhs=wv_sb[:, kt, :],
                            start=(kt == 0), stop=False)
                    nc.tensor.matmul(
                        psv, lhsT=ones_row, rhs=bv_row,
                        start=False, stop=True)
                    nc.vector.tensor_copy(
                        vaug[:, nt, :, :DH],
                        psv.rearrange("p (h d) -> p h d", d=DH))

            # ---- phase 3+4: banded attention, fused projection + store ----
            strips = {}

            def emit_strip(h, kj):
                lo, hi = max(0, kj - 1), min(NT - 1, kj + 1)
                w = (hi - lo + 1) * 128
                kjs = slice(kj * 128, (kj + 1) * 128)
                sps = slice(lo * 128, (hi + 1) * 128)
                pe = ps_e.tile([128, 384], F32, tag="pe", name="pe")
                if h < 2:
                    hs = slice(32 * h, 32 * h + 32)
                    nc.tensor.matmul(
                        pe[:, :w], lhsT=k8[hs, :, kjs], rhs=q8[hs, :, sps],
                        start=True, stop=False, perf_mode=DR,
                        skip_group_check=True)
                else:
                    i = h - 2
                    nc.tensor.matmul(
                        pe[:, :w], lhsT=k8[64:128, i, kjs],
                        rhs=q8[64:128, i, sps],
                        start=True, stop=False, skip_group_check=True)
                # band-mask accumulation: identity (e5m2) x mask panels
                panels = []
                if kj > 0:   # U panel sits where queries tile kj-1 lives
                    panels.append(((kj - 1 - lo) * 128, 0))
                if kj < NT - 1:  # L panel at queries tile kj+1
                    panels.append(((kj + 1 - lo) * 128, 128))
                for n_, (po, mo) in enumerate(panels):
                    nc.tensor.matmul(
                        pe[:, po:po + 128],
                        lhsT=idz[:],
                        rhs=msk[:, :, mo:mo + 128],
                        start=False, stop=(n_ == len(panels) - 1),
                        perf_mode=DR, skip_group_check=True)
                st = strip_pool.tile([128, 384], BF16, tag="strip", name="st")
                nc.scalar.activation(
                    st[:, :w], pe[:, :w],
                    mybir.ActivationFunctionType.Exp, scale=SCALE)
                strips[(h, kj)] = (st, lo)

            def process_tile(t):
                ts_ = slice(t * 128, (t + 1) * 128)
                ks = [k for k in (t - 1, t, t + 1) if 0 <= k < NT]
                pu = ps_u.tile([128, HPC, DH + 1], F32, tag="pu", name="pu")
                # PE executes in emission order, so the first matmul's
                # start=True zeroes the whole 2KB bank region before the
                # other 11 accumulate into their sub-slices.
                for h in range(HPC):
                    for i, k2 in enumerate(ks):
                        st, lo2 = strips[(h, k2)]
                        col = (t - lo2) * 128
                        nc.tensor.matmul(
                            pu[:, h, :], lhsT=st[:, col:col + 128],
                            rhs=vaug[:, k2, h, :],
                            start=(h == 0 and i == 0),
                            stop=(h == HPC - 1 and i == len(ks) - 1),
                            skip_group_check=True)
                rec = small.tile([128, HPC], F32, tag="rec", name="rec")
                nc.vector.reciprocal(rec[:], pu[:, :, DH])
                ao = small.tile([128, HPC, DH], BF16, tag="ao", name="ao")
                nc.vector.tensor_tensor(
                    ao[:], pu[:, :, :DH],
                    rec[:].unsqueeze(2).broadcast_to([128, HPC, DH]),
                    mybir.AluOpType.mult)
                ptb = ps_t.tile([128, 2, 128], BF16, tag="pt", name="ptb")
                for g in range(2):
                    nc.tensor.transpose(
                        ptb[:, g, :], ao[:, 2 * g:2 * g + 2, :], ident[:])
                att2 = att2p.tile([128, 2, 128], BF16, tag="att2", name="att2")
                nc.vector.tensor_copy(
                    att2.rearrange("p g n -> p (g n)"),
                    ptb.rearrange("p g n -> p (g n)"))
                # fused output projection for this token tile
                y_sb = io.tile([128, E], F16, tag="y", name="y_sb")
                for fc in range(2):
                    ps = ps_mm.tile([128, 2, 256], F32, tag="mm", name="ps_y")
                    psy = ps.rearrange("p a b -> p (a b)")
                    fs = slice(fc * 512, (fc + 1) * 512)
                    for g in range(2):
                        nc.tensor.matmul(
                            psy,
                            lhsT=att2[:, g, :],
                            rhs=wp_sb[:, g, fs],
                            start=(g == 0), stop=(g == 1))
                    if fc == 0:
                        nc.scalar.activation(
                            y_sb[:, fs], psy,
                            mybir.ActivationFunctionType.Copy)
                    else:
                        nc.vector.tensor_copy(y_sb[:, fs], psy)
                nc.sync.dma_start(y_d[ts_, :], y_sb[:])

            # interleave: strips/tiles ride along the qkv chunks so exp (Act)
            # overlaps projection matmuls (PE).  strip kj needs tokens up to
            # 128*(kj+2) <= 512*(ch+1) => kj <= 4*ch+2.
            LEAD = 2
            CHUNK_STRIPS = [range(0, 3), range(3, 7), range(7, 11),
                            range(11, 16)]
            for ch in range(NCH):
                emit_qkv_chunk(ch)
                for kj in CHUNK_STRIPS[ch]:
                    for h in range(HPC):
                        emit_strip(h, kj)
                    if kj >= LEAD:
                        process_tile(kj - LEAD)
            for t in range(NT - LEAD, NT):
                process_tile(t)

    nc.compile()
    return nc


def _get_nc():
    global _CACHED_NC
    if _CACHED_NC is None:
        _CACHED_NC = _build_nc()
    return _CACHED_NC


def _prep_core(x_b, Wq, bq, Wk, bk, Wv, Wp, gq):
    f8 = ml_dtypes.float8_e4m3
    f8e5 = ml_dtypes.float8_e5m2
    sl = slice(SL * gq, SL * (gq + 1))
    xT = np.ascontiguousarray(x_b.T).astype(np.float32)

    # feature index (within this core's 256-slice) at (slot i, partition m):
    #   m<32: head0 dh=32i+m; 32<=m<64: head1 dh=32i+(m-32);
    #   m>=64: head (2+i), dh=m-64
    fidx = np.zeros((2, 128), np.int64)
    m = np.arange(128)
    for i in range(2):
        fidx[i, :32] = 32 * i + m[:32]
        fidx[i, 32:64] = 64 + 32 * i + (m[32:64] - 32)
        fidx[i, 64:] = 64 * (2 + i) + (m[64:] - 64)

    def qk_weight(W):
        w = np.ascontiguousarray(W[sl].T).astype(np.float32) * WSCALE
        return w[:, fidx].astype(f8)  # [E, 2, 128]

    def qk_bias(b):
        return np.asarray(b, np.float32)[sl][fidx]  # [2, 128]

    bq2, bk2 = qk_bias(bq), qk_bias(bk)
    bqk = np.stack([bq2[0], bq2[1], bk2[0], bk2[1]], axis=1)  # [128, 4]

    # band-mask panels: U keeps qcol >= p, L keeps qcol <= p
    pi = np.arange(128)
    msk = np.zeros((128, 2, 256), np.float32)
    msk[:, 0, :128] = np.where(pi[None, :] >= pi[:, None], 0.0, MASKVAL)
    msk[:, 0, 128:] = np.where(pi[None, :] <= pi[:, None], 0.0, MASKVAL)
    idz = np.zeros((128, 2, 128), np.float32)
    idz[:, 0, :] = np.eye(128, dtype=np.float32)

    return {
        "x8": xT.astype(f8),
        "xT": xT.astype(np.float16),
        "wq8": qk_weight(Wq),
        "wk8": qk_weight(Wk),
        "wv": np.ascontiguousarray(np.asarray(Wv, np.float32)[sl].T).astype(
            np.float16),
        "wp": np.ascontiguousarray(np.asarray(Wp, np.float32)[:, sl].T).astype(
            np.float16),
        "bqk": np.ascontiguousarray(bqk),
        "msk": msk.astype(f8e5),
        "idz": idz.astype(f8e5),
    }


def kernel(x, Wq, bq, Wk, bk, Wv, bv, Wp, bp):
    nc = _get_nc()
    x = np.asarray(x, np.float32)
    in_maps = []
    for c in range(8):
        b, gq = c // 4, c % 4
        m = _prep_core(x[b], np.asarray(Wq, np.float32), bq,
                       np.asarray(Wk, np.float32), bk,
                       np.asarray(Wv, np.float32),
                       np.asarray(Wp, np.float32), gq)
        sl = slice(SL * gq, SL * (gq + 1))
        aux = np.zeros((3, AUXW), np.float32)
        aux[0, :SL] = np.asarray(bv, np.float32)[sl]
        aux[1, :] = 1.0
        m["aux"] = aux.astype(ml_dtypes.bfloat16)
        in_maps.append(m)
    res = run_bass_kernel_spmd(nc, in_maps, core_ids=list(range(8)))
    ys = [res.results[c]["y"].astype(np.float32) for c in range(8)]
    bp = np.asarray(bp, np.float32)
    y = np.stack([
        ys[0] + ys[1] + ys[2] + ys[3],
        ys[4] + ys[5] + ys[6] + ys[7],
    ]) + bp[None, None, :]
    return y.astype(np.float32)


# revision 20
# speedup vs baseline: 1.2835x; 1.0191x over previous
"""Local (banded) attention kernel for Trainium2, sharded over 8 NeuronCores.

Sharding: core c handles batch b=c//4 and heads 4*(c%4)..4*(c%4)+3.
Q/K projections and QK^T run as fp8 DoubleRow matmuls (host pre-quantizes
x and the scaled Q/K weight slices, permuted so each head's 64-dim split
lands as [32 partitions x 2 DR slots]).  The band mask is accumulated into
the energy PSUM via tiny fp8e5 DoubleRow matmuls (identity stationary x
precomputed -57344 panels), so exp() needs no separate mask pass.  V and
output projections run in f16; y partials stream out in f16 and the host
sums the 4 partials per batch in f32 and adds the output bias.
"""

import ml_dtypes
import numpy as np

import concourse.bass as bass
import concourse.mybir as mybir
from concourse import bacc
from concourse.tile import TileContext
from concourse.bass_utils import run_bass_kernel_spmd
from concourse.masks import make_identity

B, N, E, H, DH, WIN = 2, 2048, 1024, 16, 64, 128
HPC = 4              # heads per core
SL = HPC * DH        # feature slice per core (256)
NT = N // 128        # 16 query/key tiles
F32 = mybir.dt.float32
F16 = mybir.dt.float16
BF16 = mybir.dt.bfloat16
F8 = mybir.dt.float8e4
F8E5 = mybir.dt.float8e5
SCALE = 1.0 / 32.0   # 1/sqrt(E)
WSCALE = 32.0        # Q/K weights are shipped as fp8(32*W); undone in copies
MASKVAL = -57344.0   # exactly representable in e5m2; /32 => -1792 pre-exp
AUXW = 264           # aux rows: 0=bv, 1=ones, 2=zeros
KO = E // 128        # 8 contraction tiles
KP = KO // 2         # 4 DoubleRow contraction-pair tiles
DR = mybir.MatmulPerfMode.DoubleRow

_CACHED_NC = None


def _build_nc():
    nc = bacc.Bacc("TRN2", target_bir_lowering=False)

    x8_d = nc.dram_tensor("x8", [E, N], F8, kind="ExternalInput")
    xT_d = nc.dram_tensor("xT", [E, N], F16, kind="ExternalInput")
    wq8_d = nc.dram_tensor("wq8", [E, 2, 128], F8, kind="ExternalInput")
    wk8_d = nc.dram_tensor("wk8", [E, 2, 128], F8, kind="ExternalInput")
    wv_d = nc.dram_tensor("wv", [E, SL], F16, kind="ExternalInput")
    wp_d = nc.dram_tensor("wp", [SL, E], F16, kind="ExternalInput")
    bqk_d = nc.dram_tensor("bqk", [128, 4], F32, kind="ExternalInput")
    aux_d = nc.dram_tensor("aux", [3, AUXW], BF16, kind="ExternalInput")
    msk_d = nc.dram_tensor("msk", [128, 2, 256], F8E5, kind="ExternalInput")
    idz_d = nc.dram_tensor("idz", [128, 2, 128], F8E5, kind="ExternalInput")
    y_d = nc.dram_tensor("y", [N, E], F16, kind="ExternalOutput")

    with TileContext(nc) as tc:
        with (
            tc.tile_pool(name="const", bufs=1) as const,
            tc.tile_pool(name="persist", bufs=1) as persist,
            tc.tile_pool(name="io", bufs=3) as io,
            tc.tile_pool(name="small", bufs=6) as small,
            tc.tile_pool(name="att2p", bufs=3) as att2p,
            tc.tile_pool(name="strips", bufs=20) as strip_pool,
            tc.tile_pool(name="ps_mm", bufs=2, space="PSUM") as ps_mm,
            tc.tile_pool(name="ps_e", bufs=2, space="PSUM") as ps_e,
            tc.tile_pool(name="ps_ut", bufs=2, space="PSUM") as ps_ut,
        ):
            # ---- small constants first (cheap DMAs) ----
            t_bv = const.tile([1, SL], BF16, name="t_bv")
            nc.sync.dma_start(t_bv[:], aux_d.ap()[0:1, :SL])
            t_ones = const.tile([1, AUXW], BF16, name="t_ones")
            nc.sync.dma_start(t_ones[:], aux_d.ap()[1:2, :])
            t_zero = const.tile([1, 128], BF16, name="t_zero")
            nc.sync.dma_start(t_zero[:], aux_d.ap()[2:3, :128])
            bv_row = t_bv[:]
            ones_row = t_ones[:, :128]
            zero_row = t_zero[:]
            rhs260 = t_ones[:, :HPC * (DH + 1)]
            bqk = const.tile([128, 4], F32)
            nc.sync.dma_start(bqk[:], bqk_d.ap())
            msk = const.tile([128, 2, 256], F8E5)
            nc.sync.dma_start(msk[:], msk_d.ap())
            idz = const.tile([128, 2, 128], F8E5)
            nc.sync.dma_start(idz[:], idz_d.ap())
            ident = const.tile([128, 128], BF16)
            make_identity(nc, ident[:])

            # ---- weights before x so compute can start early ----
            wq_sb = persist.tile([128, KP, 2, 2, 128], F8)
            nc.sync.dma_start(
                wq_sb[:], wq8_d.ap().rearrange(
                    "(kp dr p) i m -> p kp dr i m", p=128, dr=2))
            wk_sb = persist.tile([128, KP, 2, 2, 128], F8)
            nc.sync.dma_start(
                wk_sb[:], wk8_d.ap().rearrange(
                    "(kp dr p) i m -> p kp dr i m", p=128, dr=2))
            x8_sb = persist.tile([128, KO, N], F8)
            xT_sb = persist.tile([128, KO, N], F16)
            wv_sb = persist.tile([128, KO, SL], F16)
            wp_sb = persist.tile([128, 2, E], F16)
            x8_ap = x8_d.ap().rearrange("(ko p) n -> p ko n", p=128)
            xT_ap = xT_d.ap().rearrange("(ko p) n -> p ko n", p=128)
            NCH = 4
            CW = N // NCH  # 512
            # DMA order: x8 chunk 0 first so Q/K proj starts early, then wv
            # + xT chunk 0 for the V projection, wp, then remaining chunks.
            sa, sb = slice(0, 256), slice(256, CW)
            nc.sync.dma_start(x8_sb[:, :, sa], x8_ap[:, :, sa])
            nc.sync.dma_start(x8_sb[:, :, sb], x8_ap[:, :, sb])
            nc.sync.dma_start(
                wv_sb[:], wv_d.ap().rearrange("(ko p) m -> p ko m", p=128))
            s0 = slice(0, CW)
            nc.sync.dma_start(xT_sb[:, :, s0], xT_ap[:, :, s0])
            nc.sync.dma_start(
                wp_sb[:], wp_d.ap().rearrange("(g p) f -> p g f", p=128))
            for c4 in range(1, NCH):
                s = slice(c4 * CW, (c4 + 1) * CW)
                nc.sync.dma_start(x8_sb[:, :, s], x8_ap[:, :, s])
                nc.sync.dma_start(xT_sb[:, :, s], xT_ap[:, :, s])

            # ---- projection outputs ----
            # q8/k8 layout (hw only allows AP base partitions 0/32/64):
            #   head 0: partitions  0-31, slots 0/1 = dh halves  (DoubleRow)
            #   head 1: partitions 32-63, slots 0/1 = dh halves  (DoubleRow)
            #   head 2: partitions 64-127, slot 0 = full dh      (plain fp8)
            #   head 3: partitions 64-127, slot 1 = full dh      (plain fp8)
            q8 = persist.tile([128, 2, N], F8, name="q8", tag="q8")
            k8 = persist.tile([128, 2, N], F8, name="k8", tag="k8")
            vaug = persist.tile([128, NT, HPC, DH + 1], BF16)
            nc.gpsimd.memset(vaug[:, :, :, DH], 1.0)

            # ---- phase 2: QKV per x-chunk ----
            def emit_qkv_chunk(ch):
                for w_sb, out_t, bc in ((wq_sb, q8, 0), (wk_sb, k8, 2)):
                    for c2 in range(2):
                        cs = slice(ch * CW + c2 * 256, ch * CW + c2 * 256 + 256)
                        ps = ps_mm.tile([128, 2, 256], F32, tag="mm", name="ps_qk")
                        for i in range(2):
                            for kp in range(KP):
                                nc.tensor.matmul(
                                    ps[:, i, :],
                                    lhsT=w_sb[:, kp, :, i, :],
                                    rhs=x8_sb[:, 2 * kp:2 * kp + 2, cs],
                                    start=(kp == 0), stop=(kp == KP - 1),
                                    perf_mode=DR)
                        nc.scalar.activation(
                            out_t[:, 0, cs], ps[:, 0, :],
                            mybir.ActivationFunctionType.Identity,
                            scale=1.0 / WSCALE, bias=bqk[:, bc:bc + 1])
                        nc.vector.tensor_scalar(
                            out_t[:, 1, cs], ps[:, 1, :],
                            1.0 / WSCALE, bqk[:, bc + 1:bc + 2],
                            mybir.AluOpType.mult, mybir.AluOpType.add)
                for nt in range(ch * NCH, (ch + 1) * NCH):
                    ps = ps_mm.tile([128, 2, 256], F32, tag="mm", name="ps_v")
                    psv = ps[:, 0, :]
                    rs = slice(nt * 128, (nt + 1) * 128)
                    for kt in range(KO):
                        nc.tensor.matmul(
                            psv, lhsT=xT_sb[:, kt, rs], rhs=wv_sb[:, kt, :],
                            start=(kt == 0), stop=False)
                    nc.tensor.matmul(
                        psv, lhsT=ones_row, rhs=bv_row,
                        start=False, stop=True)
                    nc.vector.tensor_copy(
                        vaug[:, nt, :, :DH],
                        psv.rearrange("p (h d) -> p h d", d=DH))

            # ---- phase 3+4: banded attention, fused projection + store ----
            strips = {}

            def emit_strip_pair(hp, kj):
                """Strips for heads (2*hp, 2*hp+1) of key tile kj: QK^T +
                mask matmuls into a 2-bank psum tile, one fused exp."""
                lo, hi = max(0, kj - 1), min(NT - 1, kj + 1)
                w = (hi - lo + 1) * 128
                kjs = slice(kj * 128, (kj + 1) * 128)
                sps = slice(lo * 128, (hi + 1) * 128)
                panels = []
                if kj > 0:   # U panel sits where queries tile kj-1 lives
                    panels.append(((kj - 1 - lo) * 128, 0))
                if kj < NT - 1:  # L panel at queries tile kj+1
                    panels.append(((kj + 1 - lo) * 128, 128))
                pe2 = ps_e.tile([128, 2, 512], F32, tag="pe", name="pe2")
                for j in range(2):
                    h = 2 * hp + j
                    if h < 2:
                        hs = slice(32 * h, 32 * h + 32)
                        nc.tensor.matmul(
                            pe2[:, j, :w], lhsT=k8[hs, :, kjs],
                            rhs=q8[hs, :, sps],
                            start=True, stop=False, perf_mode=DR,
                            skip_group_check=True)
                    else:
                        i = h - 2
                        nc.tensor.matmul(
                            pe2[:, j, :w], lhsT=k8[64:128, i, kjs],
                            rhs=q8[64:128, i, sps],
                            start=True, stop=False, skip_group_check=True)
                    for n_, (po, mo) in enumerate(panels):
                        nc.tensor.matmul(
                            pe2[:, j, po:po + 128],
                            lhsT=idz[:],
                            rhs=msk[:, :, mo:mo + 128],
                            start=False, stop=(n_ == len(panels) - 1),
                            perf_mode=DR, skip_group_check=True)
                st2 = strip_pool.tile(
                    [128, 2, 384], BF16, tag="strip", name="st2")
                nc.scalar.activation(
                    st2[:, :, :w], pe2[:, :, :w],
                    mybir.ActivationFunctionType.Exp, scale=SCALE)
                strips[(2 * hp, kj)] = (st2[:, 0, :], lo)
                strips[(2 * hp + 1, kj)] = (st2[:, 1, :], lo)

            def process_tile(t):
                ts_ = slice(t * 128, (t + 1) * 128)
                ks = [k for k in (t - 1, t, t + 1) if 0 <= k < NT]
                pu = ps_ut.tile([128, HPC, DH + 1], F32, tag="ut", name="pu")
                # PE executes in emission order, so the first matmul's
                # start=True zeroes the whole 2KB bank region before the
                # other 11 accumulate into their sub-slices.
                for h in range(HPC):
                    for i, k2 in enumerate(ks):
                        st, lo2 = strips[(h, k2)]
                        col = (t - lo2) * 128
                        nc.tensor.matmul(
                            pu[:, h, :], lhsT=st[:, col:col + 128],
                            rhs=vaug[:, k2, h, :],
                            start=(h == 0 and i == 0),
                            stop=(h == HPC - 1 and i == len(ks) - 1),
                            skip_group_check=True)
                rec = small.tile([128, HPC], F32, tag="rec", name="rec")
                nc.vector.reciprocal(rec[:], pu[:, :, DH])
                ao = small.tile([128, HPC, DH], BF16, tag="ao", name="ao")
                nc.vector.tensor_tensor(
                    ao[:], pu[:, :, :DH],
                    rec[:].unsqueeze(2).broadcast_to([128, HPC, DH]),
                    mybir.AluOpType.mult)
                ptb = ps_ut.tile([128, 2, 128], BF16, tag="ut", name="ptb")
                for g in range(2):
                    nc.tensor.transpose(
                        ptb[:, g, :], ao[:, 2 * g:2 * g + 2, :], ident[:])
                att2 = att2p.tile([128, 2, 128], BF16, tag="att2", name="att2")
                nc.vector.tensor_copy(
                    att2.rearrange("p g n -> p (g n)"),
                    ptb.rearrange("p g n -> p (g n)"))
                # fused output projection for this token tile
                y_sb = io.tile([128, E], F16, tag="y", name="y_sb")
                for fc in range(2):
                    ps = ps_mm.tile([128, 2, 256], F32, tag="mm", name="ps_y")
                    psy = ps.rearrange("p a b -> p (a b)")
                    fs = slice(fc * 512, (fc + 1) * 512)
                    for g in range(2):
                        nc.tensor.matmul(
                            psy,
                            lhsT=att2[:, g, :],
                            rhs=wp_sb[:, g, fs],
                            start=(g == 0), stop=(g == 1))
                    if fc == 0:
                        nc.scalar.activation(
                            y_sb[:, fs], psy,
                            mybir.ActivationFunctionType.Copy)
                    else:
                        nc.vector.tensor_copy(y_sb[:, fs], psy)
                nc.sync.dma_start(y_d[ts_, :], y_sb[:])

            # interleave: strips/tiles ride along the qkv chunks so exp (Act)
            # overlaps projection matmuls (PE).  strip kj needs tokens up to
            # 128*(kj+2) <= 512*(ch+1) => kj <= 4*ch+2.
            LEAD = 2
            CHUNK_STRIPS = [range(0, 3), range(3, 7), range(7, 11),
                            range(11, 16)]
            for ch in range(NCH):
                emit_qkv_chunk(ch)
                for kj in CHUNK_STRIPS[ch]:
                    for hp in range(2):
                        emit_strip_pair(hp, kj)
                    if kj >= LEAD:
                        process_tile(kj - LEAD)
            for t in range(NT - LEAD, NT):
                process_tile(t)

    nc.compile()
    return nc


def _get_nc():
    global _CACHED_NC
    if _CACHED_NC is None:
        _CACHED_NC = _build_nc()
    return _CACHED_NC


def _prep_core(x_b, Wq, bq, Wk, bk, Wv, Wp, gq):
    f8 = ml_dtypes.float8_e4m3
    f8e5 = ml_dtypes.float8_e5m2
    sl = slice(SL * gq, SL * (gq + 1))
    xT = np.ascontiguousarray(x_b.T).astype(np.float32)

    # feature index (within this core's 256-slice) at (slot i, partition m):
    #   m<32: head0 dh=32i+m; 32<=m<64: head1 dh=32i+(m-32);
    #   m>=64: head (2+i), dh=m-64
    fidx = np.zeros((2, 128), np.int64)
    m = np.arange(128)
    for i in range(2):
        fidx[i, :32] = 32 * i + m[:32]
        fidx[i, 32:64] = 64 + 32 * i + (m[32:64] - 32)
        fidx[i, 64:] = 64 * (2 + i) + (m[64:] - 64)

    def qk_weight(W):
        w = np.ascontiguousarray(W[sl].T).astype(np.float32) * WSCALE
        return w[:, fidx].astype(f8)  # [E, 2, 128]

    def qk_bias(b):
        return np.asarray(b, np.float32)[sl][fidx]  # [2, 128]

    bq2, bk2 = qk_bias(bq), qk_bias(bk)
    bqk = np.stack([bq2[0], bq2[1], bk2[0], bk2[1]], axis=1)  # [128, 4]

    # band-mask panels: U keeps qcol >= p, L keeps qcol <= p
    pi = np.arange(128)
    msk = np.zeros((128, 2, 256), np.float32)
    msk[:, 0, :128] = np.where(pi[None, :] >= pi[:, None], 0.0, MASKVAL)
    msk[:, 0, 128:] = np.where(pi[None, :] <= pi[:, None], 0.0, MASKVAL)
    idz = np.zeros((128, 2, 128), np.float32)
    idz[:, 0, :] = np.eye(128, dtype=np.float32)

    return {
        "x8": xT.astype(f8),
        "xT": xT.astype(np.float16),
        "wq8": qk_weight(Wq),
        "wk8": qk_weight(Wk),
        "wv": np.ascontiguousarray(np.asarray(Wv, np.float32)[sl].T).astype(
            np.float16),
        "wp": np.ascontiguousarray(np.asarray(Wp, np.float32)[:, sl].T).astype(
            np.float16),
        "bqk": np.ascontiguousarray(bqk),
        "msk": msk.astype(f8e5),
        "idz": idz.astype(f8e5),
    }


def kernel(x, Wq, bq, Wk, bk, Wv, bv, Wp, bp):
    nc = _get_nc()
    x = np.asarray(x, np.float32)
    in_maps = []
    for c in range(8):
        b, gq = c // 4, c % 4
        m = _prep_core(x[b], np.asarray(Wq, np.float32), bq,
                       np.asarray(Wk, np.float32), bk,
                       np.asarray(Wv, np.float32),
                       np.asarray(Wp, np.float32), gq)
        sl = slice(SL * gq, SL * (gq + 1))
        aux = np.zeros((3, AUXW), np.float32)
        aux[0, :SL] = np.asarray(bv, np.float32)[sl]
        aux[1, :] = 1.0
        m["aux"] = aux.astype(ml_dtypes.bfloat16)
        in_maps.append(m)
    res = run_bass_kernel_spmd(nc, in_maps, core_ids=list(range(8)))
    ys = [res.results[c]["y"].astype(np.float32) for c in range(8)]
    bp = np.asarray(bp, np.float32)
    y = np.stack([
        ys[0] + ys[1] + ys[2] + ys[3],
        ys[4] + ys[5] + ys[6] + ys[7],
    ]) + bp[None, None, :]
    return y.astype(np.float32)


# revision 24
# speedup vs baseline: 1.5285x; 1.1909x over previous
"""Local (banded) attention kernel for Trainium2, sharded over 8 NeuronCores.

Sharding: core c handles batch b=c//4 and heads 4*(c%4)..4*(c%4)+3.
Q/K projections and QK^T run as fp8 DoubleRow matmuls (host pre-quantizes
x and the scaled Q/K weight slices, permuted so each head's 64-dim split
lands as [32 partitions x 2 DR slots]).  The band mask is accumulated into
the energy PSUM via tiny fp8e5 DoubleRow matmuls (identity stationary x
precomputed -57344 panels), so exp() needs no separate mask pass.  V and
output projections run in f16; y partials stream out in f16 and the host
sums the 4 partials per batch in f32 and adds the output bias.
"""

import ml_dtypes
import numpy as np

import concourse.bass as bass
import concourse.mybir as mybir
from concourse import bacc
from concourse.tile import TileContext
from concourse.bass_utils import run_bass_kernel_spmd
from concourse.masks import make_identity

B, N, E, H, DH, WIN = 2, 2048, 1024, 16, 64, 128
HPC = 4              # heads per core
SL = HPC * DH        # feature slice per core (256)
NT = N // 128        # 16 query/key tiles
F32 = mybir.dt.float32
F16 = mybir.dt.float16
BF16 = mybir.dt.bfloat16
F8 = mybir.dt.float8e4
F8E5 = mybir.dt.float8e5
SCALE = 1.0 / 32.0   # 1/sqrt(E)
WSCALE = 32.0        # Q/K weights are shipped as fp8(32*W); undone in copies
MASKVAL = -57344.0   # exactly representable in e5m2; /32 => -1792 pre-exp
AUXW = 264           # aux rows: 0=bv, 1=ones, 2=zeros
KO = E // 128        # 8 contraction tiles
KP = KO // 2         # 4 DoubleRow contraction-pair tiles
DR = mybir.MatmulPerfMode.DoubleRow

_CACHED_NC = None


def _build_nc():
    nc = bacc.Bacc("TRN2", target_bir_lowering=False)

    x8_d = nc.dram_tensor("x8", [E, N], F8, kind="ExternalInput")
    xT_d = nc.dram_tensor("xT", [E, N], F16, kind="ExternalInput")
    wq8_d = nc.dram_tensor("wq8", [E, 2, 128], F8, kind="ExternalInput")
    wk8_d = nc.dram_tensor("wk8", [E, 2, 128], F8, kind="ExternalInput")
    wv_d = nc.dram_tensor("wv", [E, SL], F16, kind="ExternalInput")
    wp_d = nc.dram_tensor("wp", [SL, E], F16, kind="ExternalInput")
    bqk_d = nc.dram_tensor("bqk", [128, 4], F32, kind="ExternalInput")
    aux_d = nc.dram_tensor("aux", [3, AUXW], BF16, kind="ExternalInput")
    msk_d = nc.dram_tensor("msk", [128, 2, 256], F8E5, kind="ExternalInput")
    idz_d = nc.dram_tensor("idz", [128, 2, 128], F8E5, kind="ExternalInput")
    y_d = nc.dram_tensor("y", [N, E], F16, kind="ExternalOutput")

    with TileContext(nc) as tc:
        with (
            tc.tile_pool(name="const", bufs=1) as const,
            tc.tile_pool(name="persist", bufs=1) as persist,
            tc.tile_pool(name="io", bufs=3) as io,
            tc.tile_pool(name="small", bufs=6) as small,
            tc.tile_pool(name="att2p", bufs=3) as att2p,
            tc.tile_pool(name="strips", bufs=20) as strip_pool,
            tc.tile_pool(name="ps_mm", bufs=2, space="PSUM") as ps_mm,
            tc.tile_pool(name="ps_e", bufs=2, space="PSUM") as ps_e,
            tc.tile_pool(name="ps_ut", bufs=2, space="PSUM") as ps_ut,
        ):
            # ---- DMAs ordered by first use: the Q/K path unblocks PE ----
            wq_sb = persist.tile([128, KP, 2, 2, 128], F8)
            wk_sb = persist.tile([128, KP, 2, 2, 128], F8)
            x8_sb = persist.tile([128, KO, N], F8)
            xT_sb = persist.tile([128, KO, N], F16)
            wv_sb = persist.tile([128, KO, SL], F16)
            wp_sb = persist.tile([128, 2, E], F16)
            x8_ap = x8_d.ap().rearrange("(ko p) n -> p ko n", p=128)
            xT_ap = xT_d.ap().rearrange("(ko p) n -> p ko n", p=128)
            NCH = 4
            CW = N // NCH  # 512

            sa, sb = slice(0, 256), slice(256, CW)
            nc.sync.dma_start(x8_sb[:, :, sa], x8_ap[:, :, sa])
            nc.sync.dma_start(
                wq_sb[:], wq8_d.ap().rearrange(
                    "(kp dr p) i m -> p kp dr i m", p=128, dr=2))
            nc.sync.dma_start(
                wk_sb[:], wk8_d.ap().rearrange(
                    "(kp dr p) i m -> p kp dr i m", p=128, dr=2))
            bqk = const.tile([128, 4], F32)
            nc.sync.dma_start(bqk[:], bqk_d.ap())
            nc.sync.dma_start(x8_sb[:, :, sb], x8_ap[:, :, sb])
            nc.sync.dma_start(
                wv_sb[:], wv_d.ap().rearrange("(ko p) m -> p ko m", p=128))
            t_bv = const.tile([1, SL], BF16, name="t_bv")
            nc.sync.dma_start(t_bv[:], aux_d.ap()[0:1, :SL])
            t_ones = const.tile([1, AUXW], BF16, name="t_ones")
            nc.sync.dma_start(t_ones[:], aux_d.ap()[1:2, :])
            bv_row = t_bv[:]
            ones_row = t_ones[:, :128]
            s0 = slice(0, CW)
            nc.sync.dma_start(xT_sb[:, :, s0], xT_ap[:, :, s0])
            msk = const.tile([128, 2, 256], F8E5)
            nc.sync.dma_start(msk[:], msk_d.ap())
            idz = const.tile([128, 2, 128], F8E5)
            nc.sync.dma_start(idz[:], idz_d.ap())
            ident = const.tile([128, 128], BF16)
            make_identity(nc, ident[:])
            nc.sync.dma_start(
                wp_sb[:], wp_d.ap().rearrange("(g p) f -> p g f", p=128))
            for c4 in range(1, NCH):
                s = slice(c4 * CW, (c4 + 1) * CW)
                nc.sync.dma_start(x8_sb[:, :, s], x8_ap[:, :, s])
                nc.sync.dma_start(xT_sb[:, :, s], xT_ap[:, :, s])

            # ---- projection outputs ----
            # q8/k8 layout (hw only allows AP base partitions 0/32/64):
            #   head 0: partitions  0-31, slots 0/1 = dh halves  (DoubleRow)
            #   head 1: partitions 32-63, slots 0/1 = dh halves  (DoubleRow)
            #   head 2: partitions 64-127, slot 0 = full dh      (plain fp8)
            #   head 3: partitions 64-127, slot 1 = full dh      (plain fp8)
            q8 = persist.tile([128, 2, N], F8, name="q8", tag="q8")
            k8 = persist.tile([128, 2, N], F8, name="k8", tag="k8")
            vaug = persist.tile([128, NT, HPC, DH + 1], BF16)
            nc.gpsimd.memset(vaug[:, :, :, DH], 1.0)

            # ---- phase 2: QKV emitted as drip-able units so projection
            # matmuls (PE) interleave with strip exp/copies (Act/DVE) ----
            def emit_qk_unit(ch, proj, c2):
                w_sb, out_t, bc = ((wq_sb, q8, 0), (wk_sb, k8, 2))[proj]
                cs = slice(ch * CW + c2 * 256, ch * CW + c2 * 256 + 256)
                ps = ps_mm.tile([128, 2, 256], F32, tag="mm", name="ps_qk")
                for i in range(2):
                    for kp in range(KP):
                        nc.tensor.matmul(
                            ps[:, i, :],
                            lhsT=w_sb[:, kp, :, i, :],
                            rhs=x8_sb[:, 2 * kp:2 * kp + 2, cs],
                            start=(kp == 0), stop=(kp == KP - 1),
                            perf_mode=DR)
                nc.scalar.activation(
                    out_t[:, 0, cs], ps[:, 0, :],
                    mybir.ActivationFunctionType.Identity,
                    scale=1.0 / WSCALE, bias=bqk[:, bc:bc + 1])
                nc.vector.tensor_scalar(
                    out_t[:, 1, cs], ps[:, 1, :],
                    1.0 / WSCALE, bqk[:, bc + 1:bc + 2],
                    mybir.AluOpType.mult, mybir.AluOpType.add)

            def emit_v_unit(nt):
                ps = ps_mm.tile([128, 2, 256], F32, tag="mm", name="ps_v")
                psv = ps[:, 0, :]
                rs = slice(nt * 128, (nt + 1) * 128)
                for kt in range(KO):
                    nc.tensor.matmul(
                        psv, lhsT=xT_sb[:, kt, rs], rhs=wv_sb[:, kt, :],
                        start=(kt == 0), stop=False)
                nc.tensor.matmul(
                    psv, lhsT=ones_row, rhs=bv_row,
                    start=False, stop=True)
                nc.vector.tensor_copy(
                    vaug[:, nt, :, :DH],
                    psv.rearrange("p (h d) -> p h d", d=DH))

            def chunk_units(ch):
                us = [lambda p=p, c=c: emit_qk_unit(ch, p, c)
                      for p in range(2) for c in range(2)]
                us += [lambda n=n: emit_v_unit(n)
                       for n in range(ch * NCH, (ch + 1) * NCH)]
                return us

            # ---- phase 3+4: banded attention, fused projection + store ----
            strips = {}

            def emit_strip_pair(hp, kj):
                """Strips for heads (2*hp, 2*hp+1) of key tile kj: QK^T +
                mask matmuls into a 2-bank psum tile, one fused exp."""
                lo, hi = max(0, kj - 1), min(NT - 1, kj + 1)
                w = (hi - lo + 1) * 128
                kjs = slice(kj * 128, (kj + 1) * 128)
                sps = slice(lo * 128, (hi + 1) * 128)
                panels = []
                if kj > 0:   # U panel sits where queries tile kj-1 lives
                    panels.append(((kj - 1 - lo) * 128, 0))
                if kj < NT - 1:  # L panel at queries tile kj+1
                    panels.append(((kj + 1 - lo) * 128, 128))
                pe2 = ps_e.tile([128, 2, 512], F32, tag="pe", name="pe2")
                for j in range(2):
                    h = 2 * hp + j
                    if h < 2:
                        hs = slice(32 * h, 32 * h + 32)
                        nc.tensor.matmul(
                            pe2[:, j, :w], lhsT=k8[hs, :, kjs],
                            rhs=q8[hs, :, sps],
                            start=True, stop=False, perf_mode=DR,
                            skip_group_check=True)
                    else:
                        i = h - 2
                        nc.tensor.matmul(
                            pe2[:, j, :w], lhsT=k8[64:128, i, kjs],
                            rhs=q8[64:128, i, sps],
                            start=True, stop=False, skip_group_check=True)
                    for n_, (po, mo) in enumerate(panels):
                        nc.tensor.matmul(
                            pe2[:, j, po:po + 128],
                            lhsT=idz[:],
                            rhs=msk[:, :, mo:mo + 128],
                            start=False, stop=(n_ == len(panels) - 1),
                            perf_mode=DR, skip_group_check=True)
                st2 = strip_pool.tile(
                    [128, 2, 384], BF16, tag="strip", name="st2")
                nc.scalar.activation(
                    st2[:, :, :w], pe2[:, :, :w],
                    mybir.ActivationFunctionType.Exp, scale=SCALE)
                strips[(2 * hp, kj)] = (st2[:, 0, :], lo)
                strips[(2 * hp + 1, kj)] = (st2[:, 1, :], lo)

            def process_tile(t):
                ts_ = slice(t * 128, (t + 1) * 128)
                ks = [k for k in (t - 1, t, t + 1) if 0 <= k < NT]
                pu = ps_ut.tile([128, HPC, DH + 1], F32, tag="ut", name="pu")
                # PE executes in emission order, so the first matmul's
                # start=True zeroes the whole 2KB bank region before the
                # other 11 accumulate into their sub-slices.
                for h in range(HPC):
                    for i, k2 in enumerate(ks):
                        st, lo2 = strips[(h, k2)]
                        col = (t - lo2) * 128
                        nc.tensor.matmul(
                            pu[:, h, :], lhsT=st[:, col:col + 128],
                            rhs=vaug[:, k2, h, :],
                            start=(h == 0 and i == 0),
                            stop=(h == HPC - 1 and i == len(ks) - 1),
                            skip_group_check=True)
                rec = small.tile([128, HPC], F32, tag="rec", name="rec")
                nc.vector.reciprocal(rec[:], pu[:, :, DH])
                ao = small.tile([128, HPC, DH], BF16, tag="ao", name="ao")
                nc.vector.tensor_tensor(
                    ao[:], pu[:, :, :DH],
                    rec[:].unsqueeze(2).broadcast_to([128, HPC, DH]),
                    mybir.AluOpType.mult)
                ptb = ps_ut.tile([128, 2, 128], BF16, tag="ut", name="ptb")
                for g in range(2):
                    nc.tensor.transpose(
                        ptb[:, g, :], ao[:, 2 * g:2 * g + 2, :], ident[:])
                att2 = att2p.tile([128, 2, 128], BF16, tag="att2", name="att2")
                nc.vector.tensor_copy(
                    att2.rearrange("p g n -> p (g n)"),
                    ptb.rearrange("p g n -> p (g n)"))
                # fused output projection for this token tile
                y_sb = io.tile([128, E], F16, tag="y", name="y_sb")
                for fc in range(2):
                    ps = ps_mm.tile([128, 2, 256], F32, tag="mm", name="ps_y")
                    psy = ps.rearrange("p a b -> p (a b)")
                    fs = slice(fc * 512, (fc + 1) * 512)
                    for g in range(2):
                        nc.tensor.matmul(
                            psy,
                            lhsT=att2[:, g, :],
                            rhs=wp_sb[:, g, fs],
                            start=(g == 0), stop=(g == 1))
                    if fc == 0:
                        nc.scalar.activation(
                            y_sb[:, fs], psy,
                            mybir.ActivationFunctionType.Copy)
                    else:
                        nc.vector.tensor_copy(y_sb[:, fs], psy)
                nc.sync.dma_start(y_d[ts_, :], y_sb[:])

            # interleave: chunk 0's qkv first, then per-kj strips/tile with
            # 2-3 qkv units of upcoming chunks dripped in between.  strip kj
            # needs q/k tokens up to 128*(kj+2) => all chunks < (kj+3)/4.
            LEAD = 2
            pending = []  # (chunk_idx, unit)
            for ch in range(1, NCH):
                pending.extend((ch, u) for u in chunk_units(ch))
            for u in chunk_units(0):
                u()
            done_ch = 0
            for kj in range(NT):
                # strips at kj read q/k tokens < 128*(kj+2): chunks <= (kj+1)//4
                while pending and pending[0][0] <= (kj + 1) // 4:
                    pending.pop(0)[1]()
                for hp in range(2):
                    emit_strip_pair(hp, kj)
                if kj >= LEAD:
                    process_tile(kj - LEAD)
                # steady drip: ~2 units per kj keeps PE fed between strips
                drip = 2 if kj < 12 else len(pending)
                for _ in range(min(drip, len(pending))):
                    pending.pop(0)[1]()
            for t in range(NT - LEAD, NT):
                process_tile(t)

    nc.compile()
    return nc


def _get_nc():
    global _CACHED_NC
    if _CACHED_NC is None:
        _CACHED_NC = _build_nc()
    return _CACHED_NC


def _prep_core(x_b, Wq, bq, Wk, bk, Wv, Wp, gq):
    f8 = ml_dtypes.float8_e4m3
    f8e5 = ml_dtypes.float8_e5m2
    sl = slice(SL * gq, SL * (gq + 1))
    xT = np.ascontiguousarray(x_b.T).astype(np.float32)

    # feature index (within this core's 256-slice) at (slot i, partition m):
    #   m<32: head0 dh=32i+m; 32<=m<64: head1 dh=32i+(m-32);
    #   m>=64: head (2+i), dh=m-64
    fidx = np.zeros((2, 128), np.int64)
    m = np.arange(128)
    for i in range(2):
        fidx[i, :32] = 32 * i + m[:32]
        fidx[i, 32:64] = 64 + 32 * i + (m[32:64] - 32)
        fidx[i, 64:] = 64 * (2 + i) + (m[64:] - 64)

    def qk_weight(W):
        w = np.ascontiguousarray(W[sl].T).astype(np.float32) * WSCALE
        return w[:, fidx].astype(f8)  # [E, 2, 128]

    def qk_bias(b):
        return np.asarray(b, np.float32)[sl][fidx]  # [2, 128]

    bq2, bk2 = qk_bias(bq), qk_bias(bk)
    bqk = np.stack([bq2[0], bq2[1], bk2[0], bk2[1]], axis=1)  # [128, 4]

    # band-mask panels: U keeps qcol >= p, L keeps qcol <= p
    pi = np.arange(128)
    msk = np.zeros((128, 2, 256), np.float32)
    msk[:, 0, :128] = np.where(pi[None, :] >= pi[:, None], 0.0, MASKVAL)
    msk[:, 0, 128:] = np.where(pi[None, :] <= pi[:, None], 0.0, MASKVAL)
    idz = np.zeros((128, 2, 128), np.float32)
    idz[:, 0, :] = np.eye(128, dtype=np.float32)

    return {
        "x8": xT.astype(f8),
        "xT": xT.astype(np.float16),
        "wq8": qk_weight(Wq),
        "wk8": qk_weight(Wk),
        "wv": np.ascontiguousarray(np.asarray(Wv, np.float32)[sl].T).astype(
            np.float16),
        "wp": np.ascontiguousarray(np.asarray(Wp, np.float32)[:, sl].T).astype(
            np.float16),
        "bqk": np.ascontiguousarray(bqk),
        "msk": msk.astype(f8e5),
        "idz": idz.astype(f8e5),
    }


def kernel(x, Wq, bq, Wk, bk, Wv, bv, Wp, bp):
    nc = _get_nc()
    x = np.asarray(x, np.float32)
    in_maps = []
    for c in range(8):
        b, gq = c // 4, c % 4
        m = _prep_core(x[b], np.asarray(Wq, np.float32), bq,
                       np.asarray(Wk, np.float32), bk,
                       np.asarray(Wv, np.float32),
                       np.asarray(Wp, np.float32), gq)
        sl = slice(SL * gq, SL * (gq + 1))
        aux = np.zeros((3, AUXW), np.float32)
        aux[0, :SL] = np.asarray(bv, np.float32)[sl]
        aux[1, :] = 1.0
        m["aux"] = aux.astype(ml_dtypes.bfloat16)
        in_maps.append(m)
    res = run_bass_kernel_spmd(nc, in_maps, core_ids=list(range(8)))
    ys = [res.results[c]["y"].astype(np.float32) for c in range(8)]
    bp = np.asarray(bp, np.float32)
    y = np.stack([
        ys[0] + ys[1] + ys[2] + ys[3],
        ys[4] + ys[5] + ys[6] + ys[7],
    ]) + bp[None, None, :]
    return y.astype(np.float32)
